# revision 1
# baseline (speedup 1.0000x reference)
"""Multi-head causal attention (B=2, S=2048, D=1024, H=16, Dh=64) on 8 TRN2
cores -- fp8-DoubleRow rewrite of the bf16 baseline.

Core = (b, g): batch b, head-group g (4 heads).  Band-major pipeline as the
baseline, with changes that cut PE engine time ~25% and rebalance the
elementwise work across ACT/DVE/Pool:

1. QKV projections run as fp8e4m3 DoubleRow matmuls with *scaled precision
   planes*: x host-split into (xh, 16(x-xh), x/16) and W into
   (32W, 2W, 16(32W-e4(32W))) so every plane sits mid-range in e4m3 (naive
   hi/lo splits underflow: W~0.02 is subnormal).  The DR dim1 pairing
   computes A.T@xh + B.T@xl = 32*W.T@x in one instruction per
   128-contraction: 2.67x fewer PE cycles than bf16 at ~bf16 accuracy.
2. Scores run fp8-DR with dim1 carrying K hi/lo planes (kh=e4(16K),
   kl=16K-kh) against rhs planes both = e4(Q/16): K-side exact, Q-side one
   e4m3 quantization (~1.1e-2 end-to-end, the dominant error).  Half the
   bf16 PE cost, no partition shuffles.
3. V is projected directly in [k, dh] orientation (x^T slices stationary),
   eliminating the per-band PE transposes.
4. Causal masking is folded into the score accumulation as one extra fp8-DR
   matmul per diagonal strip (identity.T @ T with T = -30 above the
   diagonal), so exp(-30+s)~0 replaces the DVE mask multiplies.
5. exp splits across ACT (native, fp16 out) and DVE/Pool via the Schraudolph
   bit trick (i16 = round(s*2^10/ln2 + (15-0.0579)*2^10), bitcast fp16) --
   bit-exact vs numpy on hardware.  Diagonal strips always go to ACT (the
   bit trick would map -30-biased scores to negative junk).
6. Input DMAs are packed (one descriptor per weight group, band-0 x slices
   first) so band 0 computes ~2us in instead of stalling ~25us on the
   one-at-a-time hardware DGE.

Attention weights and V are fp16 (same cost-model price as bf16, more
mantissa).  Softmax sums ride a ones-column in V; normalization = DVE
reciprocal + PE rank-1 broadcast + multiply-copies into the bf16 ctx.
"""

import numpy as np
import ml_dtypes

B = 2
S = 2048
D = 1024
HPC = 4
DH = 64
QB = 512
NB = S // QB
KT = 128
KD = D // 128   # 8 D-tiles
DP = KD // 2    # 4 D-pairs
N_CORES = 8

_CACHE = {}
MM_LABELS = []   # emission-order labels of tensor-engine instructions
_LAB = ["?"]

# Schraudolph fp16 exp: i16 = round(s*SA + SB), bitcast fp16.  Scores sit in
# PSUM at 128x scale (kh=e4(psum/2)=16K times q=e4(psum/4)=8Q), so the exp
# scale is 1/128.  Masked diagonal entries carry a -60*128 bias and saturate
# the int16 convert to 0x8000 = -0.0 on hardware; CoreSim wraps instead, so
# set DIAG_EXP_ACT=True to route diagonal strips to ACT when interpreting.
SSC = 1.0 / 128.0
SA = float(2.0 ** 10 / np.log(2.0))
SB = float(15.0 * 2 ** 10 - 0.0579 * 2 ** 10)
DIAG_EXP_ACT = False

OSB_PLAN = ("act", "dve")


def _build_bass():
    import concourse.bacc as bacc
    import concourse.tile as tile
    from concourse import mybir

    f32 = mybir.dt.float32
    bf16 = mybir.dt.bfloat16
    fp16 = mybir.dt.float16
    e4 = mybir.dt.float8e4
    i16 = mybir.dt.int16
    DR = mybir.MatmulPerfMode.DoubleRow
    ExpF = mybir.ActivationFunctionType.Exp
    MUL = mybir.AluOpType.mult
    ADD = mybir.AluOpType.add
    SUBT = mybir.AluOpType.subtract

    nc = bacc.Bacc("TRN2", target_bir_lowering=False)

    MM_LABELS.clear()
    _orig_mm = nc.tensor.matmul
    _orig_tr = nc.tensor.transpose

    class _TraceTensor:
        def __getattr__(self, a):
            return getattr(nc.tensor.__class__, a)

    def _mm_wrap(*a, **k):
        MM_LABELS.append(_LAB[0])
        return _orig_mm(*a, **k)
    nc.tensor.matmul = _mm_wrap

    xT2_d = nc.dram_tensor("xT2", [128, 2, KD, S], e4, kind="ExternalInput")
    xh16_d = nc.dram_tensor("xh16", [128, 2, DP, S], e4, kind="ExternalInput")
    wqk_d = nc.dram_tensor("wqk", [128, 2, KD, 512], e4, kind="ExternalInput")
    wqkC_d = nc.dram_tensor("wqkC", [128, 2, DP, 512], e4, kind="ExternalInput")
    wv_d = nc.dram_tensor("wv", [128, 2, KD, 256], e4, kind="ExternalInput")
    wvC_d = nc.dram_tensor("wvC", [128, 2, DP, 256], e4, kind="ExternalInput")
    ibt_d = nc.dram_tensor("ibt", [128, 256], bf16, kind="ExternalInput")
    wo_d = nc.dram_tensor("wo", [256, D], bf16, kind="ExternalInput")
    out_d = nc.dram_tensor("out", [S, D], bf16, kind="ExternalOutput")

    with tile.TileContext(nc) as tc:
        with (
            tc.tile_pool(name="consts", bufs=1) as consts,
            tc.tile_pool(name="persist", bufs=1) as persist,
            tc.tile_pool(name="big_psum", bufs=2, space="PSUM") as big_psum,
            tc.tile_pool(name="misc_psum", bufs=2, space="PSUM") as misc_psum,
            tc.tile_pool(name="ctx_psum", bufs=2, space="PSUM") as ctx_psum,
            tc.tile_pool(name="attn_pool", bufs=16) as attn_pool,
            tc.tile_pool(name="norm_pool", bufs=12) as norm_pool,
            tc.tile_pool(name="out_pool", bufs=8) as out_pool,
        ):
            # few big DMAs: the one-at-a-time hardware DGE costs ~625ns per
            # descriptor, so band-0 loads are consolidated per tensor
            # per-queue DMA transfers serialize, so the startup loads are
            # split across the four engine DGE queues: wqk halves race x
            # band-0 halves so the first projection starts ~2.7us in
            wqk = consts.tile([128, 2, KD, 512], e4, tag="wqk", name="wqk")
            xTa = consts.tile([128, 2, KD, S], e4, tag="xTa", name="xTa")
            xha = consts.tile([128, 2, DP, S], e4, tag="xha", name="xha")
            nc.scalar.dma_start(
                out=wqk[:, 0, :, :], in_=wqk_d[:, 0, :, :]
            )
            nc.sync.dma_start(
                out=xTa[:, 0, :, 0:QB], in_=xT2_d[:, 0, :, 0:QB]
            )
            nc.scalar.dma_start(
                out=wqk[:, 1, :, :], in_=wqk_d[:, 1, :, :]
            )
            nc.sync.dma_start(
                out=xTa[:, 1, :, 0:QB], in_=xT2_d[:, 1, :, 0:QB]
            )
            wqkC = consts.tile([128, 2, DP, 512], e4, tag="wqkC", name="wqkC")
            nc.scalar.dma_start(out=wqkC, in_=wqkC_d[:, :, :, :])
            nc.sync.dma_start(
                out=xha[:, :, :, 0:QB], in_=xh16_d[:, :, :, 0:QB]
            )
            wv = consts.tile([128, 2, KD, 256], e4, tag="wv", name="wv")
            nc.sync.dma_start(out=wv, in_=wv_d[:, :, :, :])
            wvC = consts.tile([128, 2, DP, 256], e4, tag="wvC", name="wvC")
            nc.scalar.dma_start(out=wvC, in_=wvC_d[:, :, :, :])
            xT2 = [xTa[:, :, d, :] for d in range(KD)]
            xh16 = [xha[:, :, dp, :] for dp in range(DP)]
            # ident/bias for the diagonal mask matmul (bf16: the bias is
            # -60*128, beyond fp8 range): cols 0:128 = I128, 128:256 = T
            ibt = consts.tile([128, 256], bf16, tag="ibt", name="ibt")
            nc.scalar.dma_start(out=ibt, in_=ibt_d[:, :])
            wo = []
            for p in range(2):
                t = consts.tile([128, D], bf16, tag=f"wo{p}", name=f"wo{p}")
                nc.scalar.dma_start(out=t, in_=wo_d[p * 128 : (p + 1) * 128, :])
                wo.append(t)
            # bands 1-3 of x stream in behind, spread across queues
            for qi, (lo, hi) in enumerate(((QB, 2 * QB), (2 * QB, S))):
                (nc.sync if qi == 0 else nc.scalar).dma_start(
                    out=xTa[:, :, :, lo:hi], in_=xT2_d[:, :, :, lo:hi]
                )
                (nc.scalar if qi == 0 else nc.sync).dma_start(
                    out=xha[:, :, :, lo:hi], in_=xh16_d[:, :, :, lo:hi]
                )

            qT2 = [
                persist.tile([128, 2, S], e4, tag=f"qT{p}", name=f"qT{p}")
                for p in range(2)
            ]
            kT2 = [
                persist.tile([128, 2, S], e4, tag=f"kT{p}", name=f"kT{p}")
                for p in range(2)
            ]
            v_sb = persist.tile([128, S // KT, 4, 65], fp16, tag="v", name="v")
            nc.vector.memset(v_sb[:, :, :, 64:65], 1.0)
            ctxo = [
                persist.tile([128, S], bf16, tag=f"ctxo{p}", name=f"ctxo{p}")
                for p in range(2)
            ]

            rr_state = {"exp": 0, "osb": 0}
            DMAQ = [nc.sync, nc.scalar, nc.sync, nc.scalar]

            def emit_exp(out_ap, in_ap, c=0, diag=False):
                # the two heads' exps run on different engines in parallel;
                # masked entries carry a -60*128 bias, which the Schraudolph
                # int16 convert saturates to 0x8000 = -0.0 (verified on hw)
                kind = "act" if c == 0 or (diag and DIAG_EXP_ACT) else "dve"
                if kind == "act":
                    nc.scalar.activation(
                        out=out_ap, in_=in_ap, func=ExpF, scale=SSC
                    )
                else:
                    nc.vector.tensor_scalar(
                        out=out_ap.bitcast(i16), in0=in_ap,
                        scalar1=SA * SSC, scalar2=SB, op0=MUL, op1=ADD,
                    )

            def proj_qk_chunk(j, t, p, ps=None):
                """One (t, p) Q/K projection group: 12 fp8-DR matmuls into a
                [128, QB] psum + the fp8-plane copies."""
                qsl = slice(j * QB, (j + 1) * QB)
                csl = slice(256 * t + 128 * p, 256 * t + 128 * (p + 1))
                _LAB[0] = f"proj{j}.t{t}p{p}"
                if ps is None:
                    ps = misc_psum.tile([128, QB], f32, tag="misc", name="pps")
                    for d in range(KD):
                        nc.tensor.matmul(
                            ps, lhsT=wqk[:, :, d, csl], rhs=xT2[d][:, :, qsl],
                            start=(d == 0), stop=False, perf_mode=DR,
                        )
                for dp in range(DP):
                    nc.tensor.matmul(
                        ps, lhsT=wqkC[:, :, dp, csl], rhs=xh16[dp][:, :, qsl],
                        start=False, stop=(dp == DP - 1), perf_mode=DR,
                    )
                if t == 0:
                    # both Q planes = e4(8Q) = psum/4 (Q carries the 0.125
                    # softmax scale; /512 would underflow e4m3), on the
                    # lightly-loaded ACT via activation-Copy-with-scale
                    for pl in range(2):
                        nc.scalar.activation(
                            out=qT2[p][:, pl, qsl], in_=ps,
                            func=mybir.ActivationFunctionType.Copy,
                            scale=1.0 / 4,
                        )
                else:
                    # kh = e4(16K) = e4(psum/2); kl = psum/2 - kh
                    nc.vector.tensor_scalar(
                        out=kT2[p][:, 0, qsl], in0=ps,
                        scalar1=0.5, scalar2=0.0, op0=MUL, op1=ADD,
                    )
                    nc.vector.scalar_tensor_tensor(
                        out=kT2[p][:, 1, qsl], in0=ps, scalar=0.5,
                        in1=kT2[p][:, 0, qsl], op0=MUL, op1=SUBT,
                    )

            def proj_v_chunk(j, kt4):
                """One k-tile of the V projection ([k, dh] orientation)."""
                kt = 4 * j + kt4
                ksl = slice(kt * KT, (kt + 1) * KT)
                _LAB[0] = f"projV{j}.k{kt4}"
                ps = misc_psum.tile([128, QB], f32, tag="misc", name="vps")
                for d in range(KD):
                    nc.tensor.matmul(
                        ps[:, 0:256], lhsT=xT2[d][:, :, ksl],
                        rhs=wv[:, :, d, :],
                        start=(d == 0), stop=False, perf_mode=DR,
                    )
                for dp in range(DP):
                    nc.tensor.matmul(
                        ps[:, 0:256], lhsT=xh16[dp][:, :, ksl],
                        rhs=wvC[:, :, dp, :],
                        start=False, stop=(dp == DP - 1), perf_mode=DR,
                    )
                nc.scalar.activation(
                    out=v_sb[:, kt, :, 0:64], in_=ps[:, 0:256],
                    func=mybir.ActivationFunctionType.Copy, scale=1.0 / 32,
                )

            def proj_chunks(j):
                return [
                    (lambda t=t, p=p: proj_qk_chunk(j, t, p))
                    for t in range(2) for p in range(2)
                ] + [
                    (lambda kt4=kt4: proj_v_chunk(j, kt4))
                    for kt4 in range(4)
                ]

            def emit_proj0():
                """Band 0: borrow the idle score-psum tiles so all four Q/K
                groups stay open at once, with all main-plane matmuls ahead
                of the first C-term (whose DMAs land later)."""
                bigs = [
                    big_psum.tile([128, 1024], f32, tag="big", name="pb")
                    for _ in range(2)
                ]
                pss = {
                    (t, p): bigs[t][:, p * QB : (p + 1) * QB]
                    for t in range(2) for p in range(2)
                }
                _LAB[0] = "proj0.main"
                for d in range(KD):
                    for t in range(2):
                        for p in range(2):
                            csl = slice(
                                256 * t + 128 * p, 256 * t + 128 * (p + 1)
                            )
                            nc.tensor.matmul(
                                pss[(t, p)], lhsT=wqk[:, :, d, csl],
                                rhs=xT2[d][:, :, 0:QB],
                                start=(d == 0), stop=False, perf_mode=DR,
                            )
                for t in range(2):
                    for p in range(2):
                        proj_qk_chunk(0, t, p, ps=pss[(t, p)])
                for kt4 in range(4):
                    proj_v_chunk(0, kt4)

            def emit_attention(j, fillers=()):
                """fp8-DR scores (+bias-mask on diag strips) + exp + fp16 AV.

                `fillers` are independent emission chunks (next band's
                projections, previous band's output projection) interleaved
                between i2 iterations so the PE always has backlog while the
                exp engines catch up."""
                fillers = list(fillers)
                n_slots = max(1, (4 * (j + 1) // 2) * 2)
                slot = 0

                def fill():
                    nonlocal slot
                    slot += 1
                    # proportional spread across all slots, not front-loaded
                    want = min(
                        -(-len(fillers) * slot // n_slots), len(fillers)
                    )
                    while rr_state["fidx"] < want:
                        fillers[rr_state["fidx"]]()
                        rr_state["fidx"] += 1

                rr_state["fidx"] = 0
                q0 = j * QB
                nk = 4 * (j + 1)
                for p in range(2):
                    cps = [
                        ctx_psum.tile([65, QB], f32, tag="ctx", name="ctx")
                        for _ in range(2)
                    ]
                    for i2 in range(nk // 2):
                        _LAB[0] = f"scores{j}.p{p}i{i2}"
                        sps = [
                            big_psum.tile([128, 1024], f32, tag="big", name="sps")
                            for _ in range(2)
                        ]
                        at = [
                            attn_pool.tile([128, 1024], fp16, tag="attn", name="at")
                            for _ in range(2)
                        ]
                        for half in range(2):
                            for c in range(2):
                                i = 2 * i2 + half
                                o = i - 4 * j
                                z = 128 * o if o > 0 else 0
                                ksl = slice(
                                    64 * c, 64 * c + 64
                                )
                                if o < 0:
                                    nc.tensor.matmul(
                                        sps[c][:, half * QB + z : (half + 1) * QB],
                                        lhsT=kT2[p][ksl, :, i * KT : (i + 1) * KT],
                                        rhs=qT2[p][ksl, :, q0 + z : q0 + QB],
                                        start=True, stop=True, perf_mode=DR,
                                    )
                                else:
                                    # diagonal strip [z, z+128): scores + the
                                    # causal bias I.T@T (T=-30 above diag);
                                    # the clear region [z+128, QB) is its own
                                    # accumulation group
                                    nc.tensor.matmul(
                                        sps[c][:, half * QB + z : half * QB + z + 128],
                                        lhsT=kT2[p][ksl, :, i * KT : (i + 1) * KT],
                                        rhs=qT2[p][ksl, :, q0 + z : q0 + z + 128],
                                        start=True, stop=False, perf_mode=DR,
                                    )
                                    nc.tensor.matmul(
                                        sps[c][:, half * QB + z : half * QB + z + 128],
                                        lhsT=ibt[:, 0:128],
                                        rhs=ibt[:, 128:256],
                                        start=False, stop=True,
                                    )
                                    if z + 128 < QB:
                                        nc.tensor.matmul(
                                            sps[c][
                                                :, half * QB + z + 128 : (half + 1) * QB
                                            ],
                                            lhsT=kT2[p][ksl, :, i * KT : (i + 1) * KT],
                                            rhs=qT2[p][ksl, :, q0 + z + 128 : q0 + QB],
                                            start=True, stop=True, perf_mode=DR,
                                        )
                        diag = 2 * i2 - 4 * j >= 0
                        for c in range(2):
                            if not diag:
                                if j >= 2:
                                    # big bands are exp-latency-bound: split
                                    # per half so AV starts ~500ns earlier
                                    for half in range(2):
                                        sl = slice(half * QB, (half + 1) * QB)
                                        emit_exp(
                                            at[c][:, sl], sps[c][:, sl], c=c
                                        )
                                else:
                                    emit_exp(at[c], sps[c], c=c)
                            else:
                                for half in range(2):
                                    o = 2 * i2 + half - 4 * j
                                    z = 128 * o if o > 0 else 0
                                    sl = slice(half * QB + z, (half + 1) * QB)
                                    emit_exp(
                                        at[c][:, sl], sps[c][:, sl],
                                        c=c, diag=True,
                                    )
                            h = 2 * p + c
                            _LAB[0] = f"AV{j}.p{p}i{i2}c{c}"
                            for half in range(2):
                                i = 2 * i2 + half
                                o = i - 4 * j
                                z = 128 * o if o > 0 else 0
                                nc.tensor.matmul(
                                    cps[c][:, z:QB],
                                    lhsT=v_sb[:, i, h, :],
                                    rhs=at[c][:, half * QB + z : (half + 1) * QB],
                                    start=(i == 0), stop=(i == nk - 1),
                                )
                            if c == 0:
                                fill()
                    emit_norm_p(j, p, cps)
                for f in fillers[rr_state["fidx"] :]:
                    f()

            def emit_norm_p(j, p, cps_pair):
                """ctx = cps[0:64] * broadcast(1/sums) -> bf16 ctxo, emitted
                right after p's AV so it overlaps the other p-group.  The
                broadcast runs on Pool (partition_broadcast, SBUF->SBUF) so
                the multiply has a single PSUM operand (walrus rejects
                two-PSUM-input tensor ops)."""
                q0 = j * QB
                last = j == NB - 1
                for c in range(2):
                    cps = cps_pair[c]
                    rr = norm_pool.tile([1, QB], f32, tag="rr", name="rr")
                    with nc.allow_low_precision(
                        reason="approx reciprocal feeds softmax normalize"
                    ):
                        nc.vector.reciprocal(out=rr, in_=cps[64:65, :])
                    rbc = norm_pool.tile([64, QB], f32, tag="rbc", name="rbc")
                    nc.gpsimd.partition_broadcast(rbc, rr)
                    if not last:
                        nc.vector.tensor_mul(
                            ctxo[p][64 * c : 64 * c + 64, q0 : q0 + QB],
                            cps[0:64, :],
                            rbc,
                        )
                    else:
                        # last band: per-128-col chunks so each outproj
                        # m-tile starts as soon as its columns land
                        for mc in range(4):
                            sl = slice(mc * KT, (mc + 1) * KT)
                            nc.vector.tensor_mul(
                                ctxo[p][
                                    64 * c : 64 * c + 64,
                                    q0 + mc * KT : q0 + (mc + 1) * KT,
                                ],
                                cps[0:64, sl],
                                rbc[:, sl],
                            )

            def emit_outproj_m(m, last):
                _LAB[0] = f"outproj.m{m}"
                osb = out_pool.tile([128, 1024], bf16, tag="osb", name="osb")
                for n in range(2):
                    ops = misc_psum.tile([128, QB], f32, tag="misc", name="ops")
                    for p in range(2):
                        nc.tensor.matmul(
                            ops,
                            lhsT=ctxo[p][:, m * KT : (m + 1) * KT],
                            rhs=wo[p][:, n * QB : (n + 1) * QB],
                            start=(p == 0), stop=(p == 1),
                        )
                    kind = OSB_PLAN[rr_state["osb"] % len(OSB_PLAN)]
                    rr_state["osb"] += 1
                    if kind == "act":
                        nc.scalar.copy(
                            out=osb[:, n * QB : (n + 1) * QB], in_=ops
                        )
                    else:
                        eng = nc.vector if kind == "dve" else nc.gpsimd
                        eng.tensor_copy(
                            out=osb[:, n * QB : (n + 1) * QB], in_=ops
                        )
                    if last:
                        DMAQ[(m + n) % 4].dma_start(
                            out=out_d[
                                m * KT : (m + 1) * KT, n * QB : (n + 1) * QB
                            ],
                            in_=osb[:, n * QB : (n + 1) * QB],
                        )
                if not last:
                    DMAQ[m % 4].dma_start(
                        out=out_d[m * KT : (m + 1) * KT, :], in_=osb
                    )

            emit_proj0()
            prev_out = []
            for j in range(NB):
                last = j == NB - 1
                fillers = (proj_chunks(j + 1) if not last else []) + prev_out
                emit_attention(j, fillers=fillers)
                prev_out = [
                    (lambda m=m, last=last: emit_outproj_m(m, last))
                    for m in range(4 * j, 4 * j + 4)
                ]
            for f in prev_out:
                f()

    nc.compile()
    return nc


def _get_bass():
    if "nc" not in _CACHE:
        _CACHE["nc"] = _build_bass()
    return _CACHE["nc"]


def _split_x(xT):
    """x^T [1024, S] f32 -> (xT2 [128,2,KD,S], xh16 [128,2,DP,S]) e4m3."""
    E4 = ml_dtypes.float8_e4m3
    xT2 = np.empty((128, 2, KD, S), E4)
    xh16 = np.empty((128, 2, DP, S), E4)
    for d in range(KD):
        xd = xT[d * 128 : (d + 1) * 128]
        xh = xd.astype(E4)
        xT2[:, 0, d] = xh
        xT2[:, 1, d] = (16.0 * (xd - xh.astype(np.float32))).astype(E4)
        xh16[:, d % 2, d // 2] = (xd / 16.0).astype(E4)
    return xT2, xh16


def _split_w(W):
    """W [1024, C] f32 -> (w2 [128,2,8,C], wC [128,2,4,C]) e4m3 planes."""
    E4 = ml_dtypes.float8_e4m3
    C = W.shape[1]
    w2 = np.empty((128, 2, KD, C), E4)
    wC = np.empty((128, 2, DP, C), E4)
    for d in range(KD):
        Wd = 32.0 * W[d * 128 : (d + 1) * 128]
        A = Wd.astype(E4)
        w2[:, 0, d] = A
        w2[:, 1, d] = (Wd / 16.0).astype(E4)  # 2W
        wC[:, d % 2, d // 2] = (16.0 * (Wd - A.astype(np.float32))).astype(E4)
    return w2, wC


def _make_ibt():
    """ident/bias [128, 256] bf16: cols 0:128 = I128, cols 128:256 = T with
    T = -60*128*(q < k) (the 128 matches the scores-psum scale)."""
    ident = np.eye(128, dtype=np.float32)
    kk = np.arange(128)[:, None]
    qq = np.arange(128)[None, :]
    T = (-60.0 * 128.0 * (qq < kk)).astype(np.float32)
    return np.concatenate([ident, T], axis=1).astype(ml_dtypes.bfloat16)


def _make_in_maps(x, Wq, Wk, Wv, Wo):
    bf = ml_dtypes.bfloat16
    if "ibt" not in _CACHE:
        _CACHE["ibt"] = _make_ibt()
    ibt = _CACHE["ibt"]

    xsplit = []
    for b in range(B):
        xT = np.ascontiguousarray(x[b].T).astype(np.float32)
        xsplit.append(_split_x(xT))

    in_maps = []
    for core in range(N_CORES):
        b, g = divmod(core, 4)
        hs = slice(g * 256, (g + 1) * 256)
        if core < 4:
            Wqk = np.concatenate([Wq[:, hs] * 0.125, Wk[:, hs]], axis=1)
            wqk2, wqkC = _split_w(Wqk.astype(np.float32))
            wv2, wvC = _split_w(Wv[:, hs].astype(np.float32))
            shards = {
                "wqk": wqk2, "wqkC": wqkC, "wv": wv2, "wvC": wvC,
                "wo": np.ascontiguousarray(Wo[hs, :]).astype(bf),
            }
        else:
            shards = {
                k: in_maps[core - 4][k]
                for k in ("wqk", "wqkC", "wv", "wvC", "wo")
            }
        xT2, xh16 = xsplit[b]
        in_maps.append({"xT2": xT2, "xh16": xh16, "ibt": ibt, **shards})
    return in_maps


def _run(x, Wq, Wk, Wv, Wo, bo, trace=False):
    from concourse.bass_utils import run_bass_kernel_spmd

    nc = _get_bass()
    in_maps = _make_in_maps(x, Wq, Wk, Wv, Wo)
    res = run_bass_kernel_spmd(
        nc, in_maps, core_ids=list(range(N_CORES)), trace=trace
    )
    out = np.zeros((B, S, D), np.float32)
    for core in range(N_CORES):
        out[core // 4] += res.results[core]["out"].astype(np.float32)
    out += bo.astype(np.float32)
    return out, res


def kernel(x, Wq, Wk, Wv, Wo, bo):
    x, Wq, Wk, Wv, Wo, bo = (np.asarray(a) for a in (x, Wq, Wk, Wv, Wo, bo))
    out, _ = _run(x, Wq, Wk, Wv, Wo, bo, trace=False)
    return out


def kernel_traced(x, Wq, Wk, Wv, Wo, bo):
    x, Wq, Wk, Wv, Wo, bo = (np.asarray(a) for a in (x, Wq, Wk, Wv, Wo, bo))
    return _run(x, Wq, Wk, Wv, Wo, bo, trace=True)



# revision 15
# speedup vs baseline: 1.0923x; 1.0923x over previous
"""Multi-head causal attention (B=2, S=2048, D=1024, H=16, Dh=64) on 8 TRN2
cores -- flipped-AV rewrite of the fp8-DoubleRow baseline.

Core = (b, g): batch b, head-group g (4 heads).  Band-major pipeline; the
matmul cost model charges only the moving-operand columns, so every matmul is
oriented to stream its narrow side:

1. QKV projections: fp8e4m3 DoubleRow with scaled precision planes (12 DR
   matmuls per 1024-contraction), unchanged from the baseline.
2. Scores: fp8-DR, K hi/lo planes vs Q single plane (Q is e4-quantized
   anyway).  Causal masking stays folded into the score accumulation as one
   bf16 matmul per diagonal strip.
3. AV is FLIPPED: at [k, q] chunks (128x128, fp16) are the *stationary*
   operand and v [k, 64] streams -- 64 moving columns per (head, ktile,
   qtile) instead of up-to-512, halving AV PE time.  Softmax sums ride a
   separate ones-column matmul into a shared psum bank.
4. ctx lands q-major ([q, dh]), so normalization is a per-partition scaled
   psum->sbuf copy (ACT scale-AP / DVE scalar-AP) -- no partition_broadcast,
   no separate multiply.  PE 128x128 transposes (via the ibt identity) put
   ctx back in [dh, q] for the output projection; their psum lives in the
   spare space of the sums bank.
5. exp splits across ACT (native) and DVE (Schraudolph int16 bit trick) with
   a credit balancer; diagonal-strip masking saturates to -0.0 as before.
6. PSUM: 4 banks score double-buffer + 2 banks persistent ctx (one per head
   pair) + 1 bank sums/transpose + 1 bank misc (projections + outproj).
"""

import numpy as np
import ml_dtypes

B = 2
S = 2048
D = 1024
HPC = 4
DH = 64
QB = 512
NB = S // QB
KT = 128
KD = D // 128   # 8 D-tiles
DP = KD // 2    # 4 D-pairs
N_CORES = 8

_CACHE = {}
MM_LABELS = []   # emission-order labels of tensor-engine instructions
_LAB = ["?"]
DEBUG_DUMP = False

# Schraudolph fp16 exp: i16 = round(s*SA + SB), bitcast fp16.  Scores sit in
# PSUM at 128x scale (kh=e4(psum/2)=16K times q=e4(psum/4)=8Q), so the exp
# scale is 1/128.  Masked diagonal entries carry a -60*128 bias and saturate
# the int16 convert to 0x8000 = -0.0.
SSC = 1.0 / 128.0
SA = float(2.0 ** 10 / np.log(2.0))
SB = float(15.0 * 2 ** 10 - 0.0579 * 2 ** 10)

# engine-rate constants for the exp credit balancer (ns per element/partition)
_ACT_RATE = 1.0 / 1.2
_DVE_RATE = 1.0 / 0.96


def _build_bass():
    import concourse.bacc as bacc
    import concourse.tile as tile
    from concourse import mybir

    f32 = mybir.dt.float32
    bf16 = mybir.dt.bfloat16
    fp16 = mybir.dt.float16
    e4 = mybir.dt.float8e4
    i16 = mybir.dt.int16
    DR = mybir.MatmulPerfMode.DoubleRow
    ExpF = mybir.ActivationFunctionType.Exp
    CopyF = mybir.ActivationFunctionType.Copy
    MUL = mybir.AluOpType.mult
    ADD = mybir.AluOpType.add
    SUBT = mybir.AluOpType.subtract

    nc = bacc.Bacc("TRN2", target_bir_lowering=False)

    MM_LABELS.clear()
    _orig_mm = nc.tensor.matmul

    def _mm_wrap(*a, **k):
        MM_LABELS.append(_LAB[0])
        return _orig_mm(*a, **k)
    nc.tensor.matmul = _mm_wrap

    xT2_d = nc.dram_tensor("xT2", [128, 2, KD, S], e4, kind="ExternalInput")
    xh16_d = nc.dram_tensor("xh16", [128, 2, DP, S], e4, kind="ExternalInput")
    wqk_d = nc.dram_tensor("wqk", [128, 2, KD, 512], e4, kind="ExternalInput")
    wqkC_d = nc.dram_tensor("wqkC", [128, 2, DP, 512], e4, kind="ExternalInput")
    wv_d = nc.dram_tensor("wv", [128, 2, KD, 256], e4, kind="ExternalInput")
    wvC_d = nc.dram_tensor("wvC", [128, 2, DP, 256], e4, kind="ExternalInput")
    ibt_d = nc.dram_tensor("ibt", [128, 256], bf16, kind="ExternalInput")
    wo_d = nc.dram_tensor("wo", [256, D], bf16, kind="ExternalInput")
    out_d = nc.dram_tensor("out", [S, D], bf16, kind="ExternalOutput")
    if DEBUG_DUMP:
        dbg_ctxo_d = [
            nc.dram_tensor(f"dbg_ctxo{p}", [128, S], bf16, kind="ExternalOutput")
            for p in range(2)
        ]
        dbg_v_d = nc.dram_tensor("dbg_v", [128, S // KT, 4, 64], mybir.dt.float16, kind="ExternalOutput")
        dbg_q_d = nc.dram_tensor("dbg_q", [128, 2, S], e4, kind="ExternalOutput")
        dbg_k_d = nc.dram_tensor("dbg_k", [128, 2, S], e4, kind="ExternalOutput")

    with tile.TileContext(nc) as tc:
        with (
            tc.tile_pool(name="consts", bufs=1) as consts,
            tc.tile_pool(name="persist", bufs=1) as persist,
            tc.tile_pool(name="big_psum", bufs=2, space="PSUM") as big_psum,
            tc.tile_pool(name="misc_psum", bufs=2, space="PSUM") as misc_psum,
            tc.tile_pool(name="ctx_psum", bufs=1, space="PSUM") as ctx_psum,
            tc.tile_pool(name="stp_psum", bufs=1, space="PSUM") as stp_psum,
            tc.tile_pool(name="attn_pool", bufs=12) as attn_pool,
            tc.tile_pool(name="norm_pool", bufs=8) as norm_pool,
            tc.tile_pool(name="out_pool", bufs=8) as out_pool,
        ):
            # startup DMAs: consolidated per tensor, spread across queues so
            # the first projection group's operands land early
            wqk = consts.tile([128, 2, KD, 512], e4, tag="wqk", name="wqk")
            xTa = consts.tile([128, 2, KD, S], e4, tag="xTa", name="xTa")
            xha = consts.tile([128, 2, DP, S], e4, tag="xha", name="xha")
            for d in range(KD):
                nc.scalar.dma_start(out=wqk[:, :, d, :], in_=wqk_d[:, :, d, :])
                nc.sync.dma_start(
                    out=xTa[:, :, d, 0:QB], in_=xT2_d[:, :, d, 0:QB]
                )
            wqkC = consts.tile([128, 2, DP, 512], e4, tag="wqkC", name="wqkC")
            nc.scalar.dma_start(out=wqkC, in_=wqkC_d[:, :, :, :])
            nc.sync.dma_start(out=xha[:, :, :, 0:QB], in_=xh16_d[:, :, :, 0:QB])
            wv = consts.tile([128, 2, KD, 256], e4, tag="wv", name="wv")
            nc.sync.dma_start(out=wv, in_=wv_d[:, :, :, :])
            wvC = consts.tile([128, 2, DP, 256], e4, tag="wvC", name="wvC")
            nc.scalar.dma_start(out=wvC, in_=wvC_d[:, :, :, :])
            xT2 = [xTa[:, :, d, :] for d in range(KD)]
            xh16 = [xha[:, :, dp, :] for dp in range(DP)]
            # ident/bias for the diagonal mask matmul + transpose identity
            ibt = consts.tile([128, 256], bf16, tag="ibt", name="ibt")
            nc.scalar.dma_start(out=ibt, in_=ibt_d[:, :])
            wo = []
            for p in range(2):
                t = consts.tile([128, D], bf16, tag=f"wo{p}", name=f"wo{p}")
                nc.scalar.dma_start(out=t, in_=wo_d[p * 128 : (p + 1) * 128, :])
                wo.append(t)
            # bands 1-3 of x stream in behind
            for qi, (lo, hi) in enumerate(((QB, 2 * QB), (2 * QB, S))):
                (nc.sync if qi == 0 else nc.scalar).dma_start(
                    out=xTa[:, :, :, lo:hi], in_=xT2_d[:, :, :, lo:hi]
                )
                (nc.scalar if qi == 0 else nc.sync).dma_start(
                    out=xha[:, :, :, lo:hi], in_=xh16_d[:, :, :, lo:hi]
                )

            qT2 = [
                persist.tile([128, 2, S], e4, tag=f"qT{p}", name=f"qT{p}")
                for p in range(2)
            ]
            kT2 = [
                persist.tile([128, 2, S], e4, tag=f"kT{p}", name=f"kT{p}")
                for p in range(2)
            ]
            v_sb = persist.tile([128, S // KT, 4, 64], fp16, tag="v", name="v")
            ones = persist.tile([128, 1], fp16, tag="ones", name="ones")
            nc.vector.memset(ones, 1.0)
            zz = persist.tile([128, 128], fp16, tag="zz", name="zz")
            nc.vector.memset(zz, 0.0)
            zrhs = persist.tile([128, 512], fp16, tag="zrhs", name="zrhs")
            nc.vector.memset(zrhs, 0.0)
            ctxo = [
                persist.tile([128, S], bf16, tag=f"ctxo{p}", name=f"ctxo{p}")
                for p in range(2)
            ]

            # rotating PSUM state for the current (band, head-pair):
            # ctx accumulator bank + sums/transpose bank, set per p-loop
            cur = {"ctx": None, "stp": None, "tpb": None}

            rr_state = {"exp": 0.0, "dvex": 0.0, "osb": 0}
            DMAQ = [nc.sync, nc.scalar, nc.sync, nc.scalar]

            def emit_exp(out_ap, in_ap, width, force=None):
                """Route one exp tile to ACT or DVE.  The engine must be
                uniform across a (head, q-band) so the exp approximation's
                systematic error cancels between softmax numerator and
                denominator; `force` carries that routing."""
                kind = force if force is not None else "act"
                if kind == "act":
                    rr_state["exp"] += width * _ACT_RATE + 185
                    nc.scalar.activation(
                        out=out_ap, in_=in_ap, func=ExpF, scale=SSC
                    )
                else:
                    rr_state["dvex"] += width * _DVE_RATE + 125
                    nc.vector.tensor_scalar(
                        out=out_ap.bitcast(i16), in0=in_ap,
                        scalar1=SA * SSC, scalar2=SB, op0=MUL, op1=ADD,
                    )

            def charge(engine, ns):
                rr_state[engine] += ns

            def proj_qk_chunk(j, t, p, ps=None):
                """One (t, p) Q/K projection group: 12 fp8-DR matmuls into a
                [128, QB] psum + the fp8-plane copies."""
                qsl = slice(j * QB, (j + 1) * QB)
                csl = slice(256 * t + 128 * p, 256 * t + 128 * (p + 1))
                _LAB[0] = f"proj{j}.t{t}p{p}"
                if ps is None:
                    ps = misc_psum.tile([128, QB], f32, tag="misc", name="pps")
                    for d in range(KD):
                        nc.tensor.matmul(
                            ps, lhsT=wqk[:, :, d, csl], rhs=xT2[d][:, :, qsl],
                            start=(d == 0), stop=False, perf_mode=DR,
                        )
                for dp in range(DP):
                    nc.tensor.matmul(
                        ps, lhsT=wqkC[:, :, dp, csl], rhs=xh16[dp][:, :, qsl],
                        start=False, stop=(dp == DP - 1), perf_mode=DR,
                    )
                if t == 0:
                    # both Q planes = e4(8Q) = psum/4 on ACT
                    charge("exp", 2 * 512 * _ACT_RATE + 370)
                    for pl in range(2):
                        nc.scalar.activation(
                            out=qT2[p][:, pl, qsl], in_=ps,
                            func=CopyF, scale=1.0 / 4,
                        )
                else:
                    # kh = e4(16K) = e4(psum/2); kl = psum/2 - kh on DVE
                    charge("dvex", 2 * 512 * _DVE_RATE + 250)
                    nc.vector.tensor_scalar(
                        out=kT2[p][:, 0, qsl], in0=ps,
                        scalar1=0.5, scalar2=0.0, op0=MUL, op1=ADD,
                    )
                    nc.vector.scalar_tensor_tensor(
                        out=kT2[p][:, 1, qsl], in0=ps, scalar=0.5,
                        in1=kT2[p][:, 0, qsl], op0=MUL, op1=SUBT,
                    )

            def proj_v_chunk(j, kt4):
                """One k-tile of the V projection ([k, dh] orientation)."""
                kt = 4 * j + kt4
                ksl = slice(kt * KT, (kt + 1) * KT)
                _LAB[0] = f"projV{j}.k{kt4}"
                ps = misc_psum.tile([128, QB], f32, tag="misc", name="vps")
                for d in range(KD):
                    nc.tensor.matmul(
                        ps[:, 0:256], lhsT=xT2[d][:, :, ksl],
                        rhs=wv[:, :, d, :],
                        start=(d == 0), stop=False, perf_mode=DR,
                    )
                for dp in range(DP):
                    nc.tensor.matmul(
                        ps[:, 0:256], lhsT=xh16[dp][:, :, ksl],
                        rhs=wvC[:, :, dp, :],
                        start=False, stop=(dp == DP - 1), perf_mode=DR,
                    )
                charge("exp", 256 * _ACT_RATE + 185)
                nc.scalar.activation(
                    out=v_sb[:, kt, :, :], in_=ps[:, 0:256],
                    func=CopyF, scale=1.0 / 32,
                )

            def proj_chunks(j):
                return [
                    (lambda t=t, p=p: proj_qk_chunk(j, t, p))
                    for t in range(2) for p in range(2)
                ] + [
                    (lambda kt4=kt4: proj_v_chunk(j, kt4))
                    for kt4 in range(4)
                ]

            def emit_proj0():
                """Band 0: borrow the idle score-psum tiles so all four Q/K
                groups stay open at once."""
                bigs = [
                    big_psum.tile([128, 1024], f32, tag="big", name="pb")
                    for _ in range(2)
                ]
                pss = {
                    (t, p): bigs[t][:, p * QB : (p + 1) * QB]
                    for t in range(2) for p in range(2)
                }
                _LAB[0] = "proj0.main"
                for d in range(KD):
                    for t in range(2):
                        for p in range(2):
                            csl = slice(
                                256 * t + 128 * p, 256 * t + 128 * (p + 1)
                            )
                            nc.tensor.matmul(
                                pss[(t, p)], lhsT=wqk[:, :, d, csl],
                                rhs=xT2[d][:, :, 0:QB],
                                start=(d == 0), stop=False, perf_mode=DR,
                            )
                for t in range(2):
                    for p in range(2):
                        proj_qk_chunk(0, t, p, ps=pss[(t, p)])
                for kt4 in range(4):
                    proj_v_chunk(0, kt4)

            def emit_norm_qt(j, p, qt, rr, tpv=None):
                """Normalize + transpose one q-tile of ctx for head pair p.
                rr = [128, >=2] f32 sbuf tile with reciprocals for (qt, c).
                tpv = bf16 psum view for the transpose (band<3: the sums
                bank after its group stopped; band 3: a misc tile)."""
                q0 = j * QB
                csb = norm_pool.tile([128, 2, 64], bf16, tag="csb", name="csb")
                for c in range(2):
                    charge("dvex", 64 * _DVE_RATE + 125)
                    nc.vector.tensor_scalar(
                        out=csb[:, c, :],
                        in0=cur["ctx"][:, qt * 128 + c * 64 : qt * 128 + c * 64 + 64],
                        scalar1=rr[:, c : c + 1], scalar2=None, op0=MUL,
                    )
                _LAB[0] = f"tp{j}.p{p}q{qt}"
                own_copy = tpv is None
                if own_copy:
                    tp_t = misc_psum.tile([128, QB], f32, tag="misc", name="tp_t")
                    tpv = tp_t.bitcast(bf16)
                    tsl = slice(0, 128)
                else:
                    tsl = slice(256 + 128 * qt, 384 + 128 * qt)
                for c in range(2):
                    nc.tensor.transpose(
                        tpv[64 * c : 64 * c + 64, tsl], csb[:, c, :],
                        ibt[:, 0:128],
                    )
                if own_copy:
                    charge("dvex", 128 * _DVE_RATE + 125)
                    nc.vector.tensor_copy(
                        out=ctxo[p][:, q0 + qt * 128 : q0 + (qt + 1) * 128],
                        in_=tpv[:, tsl],
                    )

            def emit_attention(j, fillers=(), tail_cb=None):
                """fp8-DR scores + exp + flipped fp16 AV.

                `fillers` are independent emission chunks interleaved between
                i2 iterations so the PE always has backlog.  `tail_cb(p, qt)`
                is called after each q-tile's normalize (last band: chains the
                output projection)."""
                fillers = list(fillers)
                nk = 4 * (j + 1)
                n_slots = max(1, nk)
                slot = 0

                def fill():
                    nonlocal slot
                    slot += 1
                    want = min(
                        -(-len(fillers) * slot // n_slots), len(fillers)
                    )
                    while rr_state["fidx"] < want:
                        fillers[rr_state["fidx"]]()
                        rr_state["fidx"] += 1

                rr_state["fidx"] = 0
                q0 = j * QB
                for p in range(2):
                    done_qt = [False] * 4
                    # per-(band, head) exp engine assignment: uniform within
                    # a head's band (softmax-consistency), balanced by credit
                    if rr_state["exp"] <= rr_state["dvex"]:
                        ckind = ("act", "dve")
                    else:
                        ckind = ("dve", "act")
                    cur["ctx"] = ctx_psum.tile(
                        [128, QB], f32, tag="ctx", name="ctx_ps"
                    )
                    cur["stp"] = stp_psum.tile(
                        [128, QB], f32, tag="stp", name="sums_tp"
                    )
                    # one accumulation group per PSUM bank: the psum "zero
                    # region" is the whole 2KB bank, so open it with a
                    # bank-covering zero matmul; all AV matmuls accumulate
                    # into it with start=False
                    _LAB[0] = f"clr{j}.p{p}"
                    nc.tensor.matmul(
                        cur["ctx"], lhsT=zz, rhs=zrhs,
                        start=True, stop=False,
                    )
                    nc.tensor.matmul(
                        cur["stp"][:, 0:16], lhsT=zz, rhs=zrhs[:, 0:16],
                        start=True, stop=False,
                    )

                    def emit_av(i2, at, p=p):
                        """AV + sums matmuls for both heads of pair p at i2,
                        plus the normalize of any q-tile pair that completes.
                        Emitted one i2 late so PE can run the next scores
                        while the exp engines produce this i2's at tiles."""
                        nk_ = 4 * (j + 1)
                        for c in range(2):
                            h = 2 * p + c
                            _LAB[0] = f"AV{j}.p{p}i{i2}c{c}"
                            for half in range(2):
                                i = 2 * i2 + half
                                o = i - 4 * j
                                for qt in range(max(o, 0), 4):
                                    atc = at[c][
                                        :, half * QB + qt * 128 : half * QB + (qt + 1) * 128
                                    ]
                                    fin = (
                                        i2 == nk_ // 2 - 1
                                        and c == 1 and half == 1 and qt == 3
                                    )
                                    nc.tensor.matmul(
                                        cur["ctx"][:, qt * 128 + c * 64 : qt * 128 + c * 64 + 64],
                                        lhsT=atc,
                                        rhs=v_sb[:, i, h, :],
                                        start=False, stop=fin,
                                    )
                                    nc.tensor.matmul(
                                        cur["stp"][:, qt * 2 + c : qt * 2 + c + 1],
                                        lhsT=atc,
                                        rhs=ones,
                                        start=False, stop=fin,
                                    )
                        if tail_cb is not None:
                            # last band: normalize each q-tile pair as soon
                            # as it completes to shorten the output tail
                            for qt0 in (0, 2):
                                if done_qt[qt0] or 2 * i2 + 1 < 4 * j + qt0 + 1:
                                    continue
                                done_qt[qt0] = True
                                rr = norm_pool.tile(
                                    [128, 4], f32, tag="rr", name="rr"
                                )
                                charge("dvex", 4 * _DVE_RATE + 125)
                                with nc.allow_low_precision(
                                    reason="approx recip feeds softmax scale"
                                ):
                                    nc.vector.reciprocal(
                                        out=rr,
                                        in_=cur["stp"][:, qt0 * 2 : qt0 * 2 + 4],
                                    )
                                for dq in range(2):
                                    emit_norm_qt(j, p, qt0 + dq, rr[:, 2 * dq :])
                                    tail_cb(p, qt0 + dq)

                    pend_av = None
                    for i2 in range(nk // 2):
                        _LAB[0] = f"scores{j}.p{p}i{i2}"
                        sps = [
                            big_psum.tile([128, 1024], f32, tag="big", name="sps")
                            for _ in range(2)
                        ]
                        at = [
                            attn_pool.tile([128, 1024], fp16, tag="attn", name="at")
                            for _ in range(2)
                        ]
                        for half in range(2):
                            for c in range(2):
                                i = 2 * i2 + half
                                o = i - 4 * j
                                z = 128 * o if o > 0 else 0
                                ksl = slice(64 * c, 64 * c + 64)
                                if o < 0:
                                    nc.tensor.matmul(
                                        sps[c][:, half * QB + z : (half + 1) * QB],
                                        lhsT=kT2[p][ksl, :, i * KT : (i + 1) * KT],
                                        rhs=qT2[p][ksl, :, q0 + z : q0 + QB],
                                        start=True, stop=True, perf_mode=DR,
                                    )
                                else:
                                    # diagonal strip [z, z+128): scores + the
                                    # causal bias I.T@T (T=-30 above diag)
                                    nc.tensor.matmul(
                                        sps[c][:, half * QB + z : half * QB + z + 128],
                                        lhsT=kT2[p][ksl, :, i * KT : (i + 1) * KT],
                                        rhs=qT2[p][ksl, :, q0 + z : q0 + z + 128],
                                        start=True, stop=False, perf_mode=DR,
                                    )
                                    nc.tensor.matmul(
                                        sps[c][:, half * QB + z : half * QB + z + 128],
                                        lhsT=ibt[:, 0:128],
                                        rhs=ibt[:, 128:256],
                                        start=False, stop=True,
                                    )
                                    if z + 128 < QB:
                                        nc.tensor.matmul(
                                            sps[c][
                                                :, half * QB + z + 128 : (half + 1) * QB
                                            ],
                                            lhsT=kT2[p][ksl, :, i * KT : (i + 1) * KT],
                                            rhs=qT2[p][ksl, :, q0 + z + 128 : q0 + QB],
                                            start=True, stop=True, perf_mode=DR,
                                        )
                        diag = 2 * i2 - 4 * j >= 0
                        for c in range(2):
                            kind = ckind[c]
                            if not diag:
                                # split per half so AV starts earlier
                                for half in range(2):
                                    sl = slice(half * QB, (half + 1) * QB)
                                    emit_exp(
                                        at[c][:, sl], sps[c][:, sl], QB,
                                        force=kind,
                                    )
                            else:
                                for half in range(2):
                                    o = 2 * i2 + half - 4 * j
                                    z = 128 * o if o > 0 else 0
                                    sl = slice(half * QB + z, (half + 1) * QB)
                                    emit_exp(
                                        at[c][:, sl], sps[c][:, sl], QB - z,
                                        force=kind,
                                    )
                            if c == 0:
                                fill()
                        if pend_av is not None:
                            pend_av()
                        pend_av = (lambda i2=i2, at=at: emit_av(i2, at))
                    pend_av()
                    if tail_cb is None:
                        # batched normalize: sums group is stopped, so the
                        # transposes may share the sums bank; one big
                        # psum->ctxo copy per head pair
                        rr = norm_pool.tile([128, 8], f32, tag="rr8", name="rr")
                        charge("dvex", 8 * _DVE_RATE + 125)
                        with nc.allow_low_precision(
                            reason="approx recip feeds softmax scale"
                        ):
                            nc.vector.reciprocal(out=rr, in_=cur["stp"][:, 0:8])
                        tpv = cur["stp"].bitcast(bf16)
                        for qt in range(4):
                            emit_norm_qt(j, p, qt, rr[:, 2 * qt :], tpv=tpv)
                        charge("dvex", 512 * _DVE_RATE + 125)
                        nc.vector.tensor_copy(
                            out=ctxo[p][:, j * QB : (j + 1) * QB],
                            in_=tpv[:, 256 : 256 + 512],
                        )
                for f in fillers[rr_state["fidx"] :]:
                    f()

            def emit_outproj_m(m, last):
                _LAB[0] = f"outproj.m{m}"
                osb = out_pool.tile([128, 1024], bf16, tag="osb", name="osb")
                for n in range(2):
                    ops = misc_psum.tile([128, QB], f32, tag="misc", name="ops")
                    for p in range(2):
                        nc.tensor.matmul(
                            ops,
                            lhsT=ctxo[p][:, m * KT : (m + 1) * KT],
                            rhs=wo[p][:, n * QB : (n + 1) * QB],
                            start=(p == 0), stop=(p == 1),
                        )
                    charge("exp", 512 * _ACT_RATE + 185)
                    nc.scalar.copy(out=osb[:, n * QB : (n + 1) * QB], in_=ops)
                    if last:
                        DMAQ[(m + n) % 4].dma_start(
                            out=out_d[
                                m * KT : (m + 1) * KT, n * QB : (n + 1) * QB
                            ],
                            in_=osb[:, n * QB : (n + 1) * QB],
                        )
                if not last:
                    DMAQ[m % 4].dma_start(
                        out=out_d[m * KT : (m + 1) * KT, :], in_=osb
                    )

            _LAB[0] = "warmup"
            wps = misc_psum.tile([128, QB], f32, tag="misc", name="wps")
            for w in range(26):
                nc.tensor.matmul(
                    wps[:, 0:256], lhsT=zz, rhs=zrhs[:, 0:256],
                    start=True, stop=True,
                )
            emit_proj0()
            prev_out = []
            for j in range(NB):
                last = j == NB - 1
                fillers = (proj_chunks(j + 1) if not last else []) + prev_out

                if last:
                    # chain each output-projection m-tile right after its
                    # q-tile's ctx lands (p=1 is the later head pair)
                    def tail_cb(p, qt, j=j):
                        if p == 1:
                            emit_outproj_m(4 * j + qt, True)
                else:
                    tail_cb = None
                emit_attention(j, fillers=fillers, tail_cb=tail_cb)
                if not last:
                    prev_out = [
                        (lambda m=m: emit_outproj_m(m, False))
                        for m in range(4 * j, 4 * j + 4)
                    ]

        if DEBUG_DUMP:
            with tc.tile_pool(name="dbgp", bufs=1) as _dbgp:
                for p in range(2):
                    nc.vector.dma_start(out=dbg_ctxo_d[:, :], in_=ctxo[p]) if False else None
                nc.sync.dma_start(out=dbg_ctxo_d[0][:, :], in_=ctxo[0])
                nc.sync.dma_start(out=dbg_ctxo_d[1][:, :], in_=ctxo[1])
                nc.sync.dma_start(out=dbg_v_d[:, :, :, :], in_=v_sb)
                nc.sync.dma_start(out=dbg_q_d[:, :, :], in_=qT2[0])
                nc.sync.dma_start(out=dbg_k_d[:, :, :], in_=kT2[0])

    nc.compile()
    return nc


def _get_bass():
    if "nc" not in _CACHE:
        _CACHE["nc"] = _build_bass()
    return _CACHE["nc"]


def _split_x(xT):
    """x^T [1024, S] f32 -> (xT2 [128,2,KD,S], xh16 [128,2,DP,S]) e4m3."""
    E4 = ml_dtypes.float8_e4m3
    xT2 = np.empty((128, 2, KD, S), E4)
    xh16 = np.empty((128, 2, DP, S), E4)
    for d in range(KD):
        xd = xT[d * 128 : (d + 1) * 128]
        xh = xd.astype(E4)
        xT2[:, 0, d] = xh
        xT2[:, 1, d] = (16.0 * (xd - xh.astype(np.float32))).astype(E4)
        xh16[:, d % 2, d // 2] = (xd / 16.0).astype(E4)
    return xT2, xh16


def _split_w(W):
    """W [1024, C] f32 -> (w2 [128,2,8,C], wC [128,2,4,C]) e4m3 planes."""
    E4 = ml_dtypes.float8_e4m3
    C = W.shape[1]
    w2 = np.empty((128, 2, KD, C), E4)
    wC = np.empty((128, 2, DP, C), E4)
    for d in range(KD):
        Wd = 32.0 * W[d * 128 : (d + 1) * 128]
        A = Wd.astype(E4)
        w2[:, 0, d] = A
        w2[:, 1, d] = (Wd / 16.0).astype(E4)  # 2W
        wC[:, d % 2, d // 2] = (16.0 * (Wd - A.astype(np.float32))).astype(E4)
    return w2, wC


def _make_ibt():
    """ident/bias [128, 256] bf16: cols 0:128 = I128, cols 128:256 = T with
    T = -60*128*(q < k) (the 128 matches the scores-psum scale)."""
    ident = np.eye(128, dtype=np.float32)
    kk = np.arange(128)[:, None]
    qq = np.arange(128)[None, :]
    T = (-60.0 * 128.0 * (qq < kk)).astype(np.float32)
    return np.concatenate([ident, T], axis=1).astype(ml_dtypes.bfloat16)


def _make_in_maps(x, Wq, Wk, Wv, Wo):
    bf = ml_dtypes.bfloat16
    if "ibt" not in _CACHE:
        _CACHE["ibt"] = _make_ibt()
    ibt = _CACHE["ibt"]

    xsplit = []
    for b in range(B):
        xT = np.ascontiguousarray(x[b].T).astype(np.float32)
        xsplit.append(_split_x(xT))

    in_maps = []
    for core in range(N_CORES):
        b, g = divmod(core, 4)
        hs = slice(g * 256, (g + 1) * 256)
        if core < 4:
            Wqk = np.concatenate([Wq[:, hs] * 0.125, Wk[:, hs]], axis=1)
            wqk2, wqkC = _split_w(Wqk.astype(np.float32))
            wv2, wvC = _split_w(Wv[:, hs].astype(np.float32))
            shards = {
                "wqk": wqk2, "wqkC": wqkC, "wv": wv2, "wvC": wvC,
                "wo": np.ascontiguousarray(Wo[hs, :]).astype(bf),
            }
        else:
            shards = {
                k: in_maps[core - 4][k]
                for k in ("wqk", "wqkC", "wv", "wvC", "wo")
            }
        xT2, xh16 = xsplit[b]
        in_maps.append({"xT2": xT2, "xh16": xh16, "ibt": ibt, **shards})
    return in_maps


def _run(x, Wq, Wk, Wv, Wo, bo, trace=False):
    from concourse.bass_utils import run_bass_kernel_spmd

    nc = _get_bass()
    in_maps = _make_in_maps(x, Wq, Wk, Wv, Wo)
    res = run_bass_kernel_spmd(
        nc, in_maps, core_ids=list(range(N_CORES)), trace=trace
    )
    out = np.zeros((B, S, D), np.float32)
    for core in range(N_CORES):
        out[core // 4] += res.results[core]["out"].astype(np.float32)
    out += bo.astype(np.float32)
    return out, res


def kernel(x, Wq, Wk, Wv, Wo, bo):
    x, Wq, Wk, Wv, Wo, bo = (np.asarray(a) for a in (x, Wq, Wk, Wv, Wo, bo))
    out, _ = _run(x, Wq, Wk, Wv, Wo, bo, trace=False)
    return out


def kernel_traced(x, Wq, Wk, Wv, Wo, bo):
    x, Wq, Wk, Wv, Wo, bo = (np.asarray(a) for a in (x, Wq, Wk, Wv, Wo, bo))
    return _run(x, Wq, Wk, Wv, Wo, bo, trace=True)
